# revision 12
# baseline (speedup 1.0000x reference)
"""Trainium2 Bass kernel for nn_DNM_Conv (LayerNorm -> synapse contraction ->
dendritic weighting -> GELU -> residual multiply).

Algebraic reduction of the reference:
    y = LayerNorm(x)                                  (b, n, d)
    t[b,o,d] = sum_n W[o,n] * y[b,n,d] + c[o]
        where W[o,n] = sum_m dw[o,m]*sw[o,m,n],  c[o] = sum_{m,n} dw[o,m]*sb[o,m,n]
    out = x * (gelu_erf(t) + 1)                       (o == n == 196)

Instead of normalizing x, the per-row LN scale is folded into the weights:
    Wr[o,n] = W[o,n] * rstd[n]        (per batch, tiny 196x196 scale)
    t[o,d]  = (Wr @ x)[o,d] - (Wr @ mu)[o] + c[o]
The mu-correction rides as an extra rhs column (x tile has 769 columns, the
last one holding mu), so one widened matmul produces both t and Wr@mu.

Distribution: data-parallel over batch, 8 batches per core on 8 cores.
Datapath is fp16 end to end (x cast on host, fp32 PSUM accumulation,
fp16 output cast back on host); LN statistics are computed in fp32.
"""

import numpy as np

B, N, D, O, M = 64, 196, 768, 196, 2
N_CORES = 8
BPC = B // N_CORES          # batches per core
NPAIR = BPC // 2            # batch pairs (DMA granularity)
NA, NB = 128, 68            # n partition split
OA, OB = 128, 68            # o partition split
DC = 384                    # matmul moving free-dim chunk
LN_EPS = 1e-5

_NC_CACHE = {}


def _build_nc(nontrivial_ln):
    import concourse.bacc as bacc
    import concourse.tile as tile
    import concourse.bass as bass
    from concourse.tile import add_dep_helper
    from concourse import mybir
    from contextlib import ExitStack

    F32 = mybir.dt.float32
    F16 = mybir.dt.float16
    AF = mybir.ActivationFunctionType
    OP = mybir.AluOpType

    nc = bacc.Bacc()
    x_d = nc.declare_dram_parameter("x", [BPC, N, D], F16, isOutput=False)
    wt_d = nc.declare_dram_parameter("wt", [N, O], F16, isOutput=False)
    c_d = nc.declare_dram_parameter("c", [O, 1], F32, isOutput=False)
    if nontrivial_ln:
        lnw_d = nc.declare_dram_parameter("lnw", [1, D], F32, isOutput=False)
        lnbe_d = nc.declare_dram_parameter("lnbe", [O, D], F32, isOutput=False)
    out_d = nc.declare_dram_parameter("out", [BPC, N, D], F16, isOutput=True)

    x_pair = x_d.ap().rearrange("(q j) n d -> q n j d", j=2)    # (4, 196, 2, 768)
    out_pair = out_d.ap().rearrange("(q j) n d -> q n j d", j=2)

    with tile.TileContext(nc) as tc, ExitStack() as ctx:
        const = ctx.enter_context(tc.tile_pool(name="const", bufs=1))
        xpool = ctx.enter_context(tc.tile_pool(name="xpool", bufs=NPAIR))
        stpool = ctx.enter_context(tc.tile_pool(name="stpool", bufs=BPC))
        wrpool = ctx.enter_context(tc.tile_pool(name="wrpool", bufs=3))
        gpool = ctx.enter_context(tc.tile_pool(name="gpool", bufs=3))
        opool = ctx.enter_context(tc.tile_pool(name="opool", bufs=2))
        psum = ctx.enter_context(tc.tile_pool(name="psum", bufs=2, space="PSUM"))

        # ---- constants ----
        wt_a = const.tile([NA, O], F16, tag="wt_a")
        wt_b = const.tile([NB, O], F16, tag="wt_b")
        nc.sync.dma_start(out=wt_a[:], in_=wt_d[0:NA, :])
        nc.sync.dma_start(out=wt_b[:], in_=wt_d[NA:N, :])
        c_a = const.tile([OA, 1], F32, tag="c_a")
        c_b = const.tile([OB, 1], F32, tag="c_b")
        nc.sync.dma_start(out=c_a[:], in_=c_d[0:OA, :])
        nc.sync.dma_start(out=c_b[:], in_=c_d[OA:O, :])
        eps_t = const.tile([128, 1], F32, tag="eps")
        nc.vector.memset(eps_t[:], LN_EPS)
        if nontrivial_ln:
            lnw_t = const.tile([128, D], F32, tag="lnw")
            lnw_bcast = bass.AP(tensor=lnw_d.ap().tensor, offset=0,
                                ap=[[0, 128], [1, D]])
            nc.sync.dma_start(out=lnw_t[:], in_=lnw_bcast)
            lnbe_a = const.tile([OA, D], F32, tag="lnbe_a")
            lnbe_b = const.tile([OB, D], F32, tag="lnbe_b")
            nc.sync.dma_start(out=lnbe_a[:], in_=lnbe_d[0:OA, :])
            nc.sync.dma_start(out=lnbe_b[:], in_=lnbe_d[OA:O, :])

        nsplit = ((0, NA), (NA, NB))

        # ---- phase A: load x (paired, fp16, columns 768:770 reserved for
        #      [mean, var]), bn stats.  Also seed the output with x (the
        #      "+ x" residual term) -- the tail accumulates x*g on top. ----
        xtiles = []  # [pair][ci] -> (pn, 2, 770) fp16
        seed_insts = []  # [pair][ci]
        for q in range(NPAIR):
            xq = []
            for ci, (p0, pn) in enumerate(nsplit):
                xt = xpool.tile([pn, 2, D + 2], F16, tag=f"x{ci}")
                nc.sync.dma_start(out=xt[:, :, 0:D],
                                    in_=x_pair[q, p0:p0 + pn, :, :])
                xq.append(xt)
            xtiles.append(xq)
        for q in range(NPAIR):
            si = []
            for ci, (p0, pn) in enumerate(nsplit):
                ins = nc.sync.dma_start(out=out_pair[q, p0:p0 + pn, :, :],
                                        in_=xtiles[q][ci][:, :, 0:D])
                si.append(ins)
            seed_insts.append(si)
        for i in range(BPC):
            q, j = divmod(i, 2)
            for ci, (p0, pn) in enumerate(nsplit):
                xt = xtiles[q][ci]
                stats = stpool.tile([pn, 2, 6], F32, tag=f"stats{ci}")
                xg = xt[:, j, 0:D].rearrange("p (s f) -> p s f", s=2)
                for s in range(2):
                    nc.vector.bn_stats(out=stats[:, s, :], in_=xg[:, s, :])
                # write [mean, var] (fp16) straight into x columns 768:770 --
                # the mean becomes the extra matmul-rhs column, var feeds rstd
                nc.vector.bn_aggr(out=xt[:, j, D:D + 2], in_=stats[:])

        # ---- rstd for all batches (single ACT table-set period) ----
        rstds = []
        rstd_insts = []
        for i in range(BPC):
            q, j = divmod(i, 2)
            ri = []
            for ci, (p0, pn) in enumerate(nsplit):
                rstd = stpool.tile([pn, 1], F32, tag=f"rstd{ci}")
                ins = nc.scalar.activation(out=rstd[:],
                                           in_=xtiles[q][ci][:, j, D + 1:D + 2],
                                           func=AF.Abs_reciprocal_sqrt,
                                           bias=eps_t[0:pn, :], scale=1.0)
                rstd_insts.append(ins)
                ri.append(rstd)
            rstds.append(ri)

        # Wr = wt * rstd (fp16); a-tile on ACT, b-tile on DVE
        wrs = []
        for i in range(BPC):
            wri = []
            for ci, (p0, pn) in enumerate(nsplit):
                wr = wrpool.tile([pn, O], F16, tag=f"wr{ci}")
                wt_t = wt_a if ci == 0 else wt_b
                if ci == 0:
                    nc.scalar.activation(out=wr[:], in_=wt_t[:], func=AF.Copy,
                                         scale=rstds[i][ci][:])
                else:
                    nc.vector.tensor_scalar_mul(out=wr[:], in0=wt_t[:],
                                                scalar1=rstds[i][ci][:])
                wri.append(wr)
            wrs.append(wri)

        # ---- phase B: matmul + gelu (per batch), then pair-wide residual
        #      multiply + store ----
        osplit = ((0, OA, c_a), (OA, OB, c_b))
        first_gelu_logged = False
        for q in range(NPAIR):
            xs = xtiles[q]
            out_a = opool.tile([NA, 2, D], F16, tag="out0")
            out_b = opool.tile([NB, 2, D], F16, tag="out1")
            outs = (out_a, out_b)
            # g[oc][dc] spans both batches of the pair: (on, 2, DC)
            gt = [[gpool.tile([on, 2, DC], F16, tag=f"g{oc}{dc}", name=f"g{oc}{dc}")
                   for dc in range(2)] for oc, (o0, on, c_t) in enumerate(osplit)]

            for j in range(2):
                i = 2 * q + j
                for oc, (o0, on, c_t) in enumerate(osplit):
                    # dc=1 widened matmul first: cols 384..768 plus the mu column
                    pm1 = psum.tile([on, DC + 1], F32, tag=f"pm{oc}1")
                    for k, wr in enumerate(wrs[i]):
                        nc.tensor.matmul(pm1[:], wr[:, o0:o0 + on],
                                         xs[k][:, j, DC:D + 1],
                                         start=(k == 0), stop=(k == 1))
                    gbias = stpool.tile([on, 1], F32, tag=f"gb{oc}")
                    nc.vector.tensor_tensor(out=gbias[:], in0=c_t[:],
                                            in1=pm1[:, DC:DC + 1],
                                            op=OP.subtract)
                    pm0 = psum.tile([on, DC], F32, tag=f"pm{oc}0")
                    for k, wr in enumerate(wrs[i]):
                        nc.tensor.matmul(pm0[:], wr[:, o0:o0 + on],
                                         xs[k][:, j, 0:DC],
                                         start=(k == 0), stop=(k == 1))

                    for dc, pm in ((1, pm1), (0, pm0)):
                        ds = slice(dc * DC, (dc + 1) * DC)
                        if nontrivial_ln:
                            lnbe_t = lnbe_a if oc == 0 else lnbe_b
                            # t = lnw * (pm - Wr@mu) + lnbe + c
                            nc.vector.tensor_scalar_sub(
                                out=pm[:, 0:DC], in0=pm[:, 0:DC],
                                scalar1=pm1[:, DC:DC + 1])
                            nc.vector.tensor_mul(out=pm[:, 0:DC],
                                                 in0=pm[:, 0:DC],
                                                 in1=lnw_t[0:on, ds])
                            nc.vector.tensor_add(out=pm[:, 0:DC],
                                                 in0=pm[:, 0:DC],
                                                 in1=lnbe_t[:, ds])
                            ins = nc.scalar.activation(
                                out=gt[oc][dc][:, j, :], in_=pm[:, 0:DC],
                                func=AF.Gelu, bias=c_t[:], scale=1.0)
                        else:
                            ins = nc.scalar.activation(
                                out=gt[oc][dc][:, j, :], in_=pm[:, 0:DC],
                                func=AF.Gelu, bias=gbias[:], scale=1.0)
                        if not first_gelu_logged:
                            first_gelu_logged = True
                            add_dep_helper(ins.ins, rstd_insts[-1].ins,
                                           sync=True,
                                           reason="sqrt-set before gelu-set")

            # pair-wide residual multiply p1 = g * x (DVE 2x-mode slabs),
            # then accumulate into the x-seeded output via gpsimd DMA
            for oc, (o0, on, c_t) in enumerate(osplit):
                for dc in range(2):
                    ds = slice(dc * DC, (dc + 1) * DC)
                    nc.vector.tensor_mul(out=outs[oc][:, :, ds],
                                         in0=gt[oc][dc][:],
                                         in1=xs[oc][:, :, ds])
            for ci, (p0, pn) in enumerate(nsplit):
                acc = nc.gpsimd.dma_start(out=out_pair[q, p0:p0 + pn, :, :],
                                          in_=outs[ci][:],
                                          accum_op=OP.add)
                add_dep_helper(acc.ins, seed_insts[q][ci].ins, sync=True,
                               reason="accumulate after residual seed")

    nc.compile()
    return nc


def kernel(x, ln_w, ln_b, sw, sb, dw, _trace=False):
    from concourse.bass_utils import run_bass_kernel_spmd

    x = np.asarray(x, dtype=np.float32)
    ln_w = np.asarray(ln_w, dtype=np.float32)
    ln_b = np.asarray(ln_b, dtype=np.float32)
    sw = np.asarray(sw, dtype=np.float32)
    sb = np.asarray(sb, dtype=np.float32)
    dw = np.asarray(dw, dtype=np.float32)

    x16 = np.ascontiguousarray(x.astype(np.float16))

    # Fold dendritic weights into the synapse contraction (host, ~0.1 ms).
    W = np.einsum("om,omn->on", dw, sw)            # (o, n)
    WT = np.ascontiguousarray(W.T.astype(np.float16))
    c = np.einsum("om,om->o", dw, sb.sum(-1)).astype(np.float32)[:, None]

    nontrivial_ln = not (np.all(ln_w == 1.0) and np.all(ln_b == 0.0))
    key = bool(nontrivial_ln)
    if key not in _NC_CACHE:
        _NC_CACHE[key] = _build_nc(nontrivial_ln)
    nc = _NC_CACHE[key]

    in_maps = []
    for i in range(N_CORES):
        m = {"x": x16[i * BPC:(i + 1) * BPC], "wt": WT, "c": c}
        if nontrivial_ln:
            m["lnw"] = ln_w[None, :]
            m["lnbe"] = (W.sum(-1)[:, None] * ln_b[None, :]).astype(np.float32)
        in_maps.append(m)

    res = run_bass_kernel_spmd(nc, in_maps, core_ids=list(range(N_CORES)),
                               trace=_trace)
    out = np.concatenate([res.results[i]["out"] for i in range(N_CORES)],
                         axis=0).astype(np.float32)
    if _trace:
        return out, res
    return out
